# revision 3
# baseline (speedup 1.0000x reference)
"""DCRNN (K=1) fused kernel for Trainium2, 8-core data-parallel over nodes. v3.

Math: with H0=0 and K=1 (edges unused),
    xm  = x * mask
    Z'  = sigmoid(-a),  a = xm @ Wz          (Wz = w_z[0,0,:256]+w_z[1,0,:256])
    Ht  = tanh(xm @ Wh)
    m   = Z' * Ht
    h1  = elu(m) ~= m + ((c1-1) + c2*n)*n,  n = min(m, 0)   (poly fit on [-1,0])
    out = h1 @ w_lin.T                       (b_lin == 0 for this problem)

v3 vs v2:
  * ZPHT_ELU custom DVE op: h1 = elupoly(zp*ht) in ONE 6-stage DVE
    instruction (1 elem/lane/cycle) -- replaces the tensor_mul + 2-4 op
    elu chains of v2. DVE busy drops ~30.6us -> ~23us.
  * All biases are zero and the expanded poly has no constant term, so
    every bias-add disappears; the PSUM->SBUF move is a DVE tensor_copy.
  * All blocks use single-part finals (no alt 2-part blocks): PE drops
    ~28.8us -> ~26.3us. PE is the predicted wall.
  * Gate matmul loop is k-outer (fewer weight switches).
"""

import numpy as np

DTYPE = "float16"

CFG = {
    "io_bufs": 3,
    "ew_bufs": 3,
    "po_bufs": 2,
    "pipeline": 2,             # 2 | 3 stage software pipeline
    "move_engine": "vector",   # "vector" | "scalar"  (PSUM->SBUF out move)
    "in_dma": "sync",
    "out_dma": "gpsimd",
    "k_outer": True,
    "blocks_plan": None,       # None -> [896]*7
    "dma_div": 1,              # TIMING PROBE ONLY: divide input DMA bytes
    "gate_k": 2,               # TIMING PROBE ONLY: 1 -> halve gate PE work
    "act_div": 1,              # TIMING PROBE ONLY: 2 -> halve ACT work
    "zpht_plain": False,       # TIMING PROBE ONLY: plain mul instead of fused
    "col_split": True,         # concurrent A/B finals via PE column tiling
    "merged768": False,        # 768-blocks, per-gate merged psum, 2 ACT/block
}

N_FULL = 50000
C_IN = 256
C_HID = 256
C_OUT = 64
N_CORES = 8
PER_CORE = 6272
N_PAD = PER_CORE * N_CORES

# elu(m)-m on [-1,0]: exp(n)-1-n ~= (c1-1)*n + c2*n^2  (least-squares fit)
POLY_C1 = 0.93055057
POLY_C2 = 0.30871908

_module_cache = {}
_zpht_elu_op = None


def _get_zpht_elu_op():
    """Register the fused h1 = elupoly(zp*ht) custom DVE op (idempotent)."""
    global _zpht_elu_op
    if _zpht_elu_op is not None:
        return _zpht_elu_op
    import concourse.dve_ops as dve_ops
    from concourse.dve_spec import Spec, Src0, Src1, C0, C1, Zero, minn, lower
    from concourse.dve_uop import DveOpSpec

    m = Src0 * Src1
    n = minn(m, Zero)
    body = m + (n * C0 + C1) * n

    def _ref(in0, in1, s0, s1, imm2):
        mm = in0.astype(np.float32) * in1.astype(np.float32)
        nn = np.minimum(mm, 0.0)
        return (mm + (nn * s0 + s1) * nn).astype(np.float32)

    spec = Spec(body=body, reference=_ref)
    name = "ZPHT_ELU"
    if name in dve_ops._SUB_OPCODE_FOR_NAME:
        _zpht_elu_op = next(op for op in dve_ops.OPS if op.name == name)
        return _zpht_elu_op
    row = dve_ops._CUSTOM_DVE_ROW_BASE + len(dve_ops.OPS)
    assert row < 0x20
    # compute the sha for this environment rather than hardcoding
    shas = {}
    for ver in ("v3", "v4"):
        uops = lower(spec, ver=ver)
        shas[ver] = DveOpSpec(name=name, opcode=row, uops=uops, rd1_en=True).sha(ver)
    op = dve_ops.DveOp(name, spec, subdim=False, uops_sha=shas)
    dve_ops.OPS.append(op)
    dve_ops.CUSTOM_DVE_SPECS[name] = spec
    dve_ops._SUB_OPCODE_FOR_NAME[name] = row
    _zpht_elu_op = op
    return op


def _blocks_for_cfg(cfg):
    if cfg.get("blocks_plan"):
        return cfg["blocks_plan"]
    if cfg.get("merged768"):
        return [768] * 8 + [128]
    return [896] * (PER_CORE // 896)


def _build_module(dtype_name, cfg=None, repeat=1):
    import concourse.bacc as bacc
    import concourse.tile as tile
    import concourse.mybir as mybir

    cfg = dict(CFG, **(cfg or {}))
    zpht_op = _get_zpht_elu_op()
    f32 = mybir.dt.float32
    cdt = {
        "float32": mybir.dt.float32,
        "float16": mybir.dt.float16,
        "bfloat16": mybir.dt.bfloat16,
    }[dtype_name]
    AF = mybir.ActivationFunctionType

    nc = bacc.Bacc("TRN2", target_bir_lowering=False, debug=False)

    x_t = nc.declare_dram_parameter("x_t", [2, 128, PER_CORE], cdt, isOutput=False)
    mk_t = nc.declare_dram_parameter("mk_t", [2, 128, PER_CORE], cdt, isOutput=False)
    wz_t = nc.declare_dram_parameter("wz_t", [2, 128, C_HID], cdt, isOutput=False)
    wh_t = nc.declare_dram_parameter("wh_t", [2, 128, C_HID], cdt, isOutput=False)
    # packed final weights: [k, 128, 128]; [:, :, 0:64]=Wl (chunk A) / 0 (B)
    if cfg["col_split"]:
        wl_t = nc.declare_dram_parameter("wl_t", [2, 128, C_OUT], cdt,
                                         isOutput=False)
    else:
        wlA_t = nc.declare_dram_parameter("wlA_t", [2, 128, 128], cdt,
                                          isOutput=False)
        wlB_t = nc.declare_dram_parameter("wlB_t", [2, 128, 128], cdt,
                                          isOutput=False)
    out_t = nc.declare_dram_parameter("out_t", [128, PER_CORE // 2], cdt,
                                      isOutput=True)

    x_v = x_t.ap().rearrange("k p n -> p k n")
    mk_v = mk_t.ap().rearrange("k p n -> p k n")

    blocks = _blocks_for_cfg(cfg)
    assert sum(blocks) == PER_CORE and all(b % 2 == 0 for b in blocks), blocks

    with tile.TileContext(nc) as tc:
        with (
            tc.tile_pool(name="consts", bufs=1) as consts,
            tc.tile_pool(name="io", bufs=cfg["io_bufs"]) as io,
            tc.tile_pool(name="ew", bufs=cfg["ew_bufs"]) as ew,
            tc.tile_pool(name="outs", bufs=3) as outs,
            tc.tile_pool(name="gpsum", bufs=3, space="PSUM") as gpsum,
            tc.tile_pool(name="opsum", bufs=cfg["po_bufs"], space="PSUM") as opsum,
        ):
            eng = {"vector": nc.vector, "gpsimd": nc.gpsimd,
                   "sync": nc.sync, "scalar": nc.scalar}
            in_dma = eng[cfg["in_dma"]]
            out_dma = eng[cfg["out_dma"]]

            wz_sb = consts.tile([128, 2, C_HID], cdt)
            wh_sb = consts.tile([128, 2, C_HID], cdt)
            nc.sync.dma_start(out=wz_sb[:], in_=wz_t.ap().rearrange("k p m -> p k m"))
            nc.sync.dma_start(out=wh_sb[:], in_=wh_t.ap().rearrange("k p m -> p k m"))
            if cfg["col_split"]:
                wl_sb = consts.tile([128, 2, C_OUT], cdt)
                nc.sync.dma_start(out=wl_sb[:],
                                  in_=wl_t.ap().rearrange("k p m -> p k m"))
            else:
                wlA_sb = consts.tile([128, 2, 128], cdt)
                wlB_sb = consts.tile([128, 2, 128], cdt)
                nc.sync.dma_start(out=wlA_sb[:],
                                  in_=wlA_t.ap().rearrange("k p m -> p k m"))
                nc.sync.dma_start(out=wlB_sb[:],
                                  in_=wlB_t.ap().rearrange("k p m -> p k m"))

            # Warm the activation tables (sigmoid set also holds tanh) so the
            # ~2.7us load overlaps the first input DMAs.
            warm = consts.tile([1, 2], f32)
            nc.vector.memset(warm[:, 0:1], 0.0)
            nc.scalar.activation(warm[:, 0:1], warm[:, 0:1], AF.Sigmoid)
            nc.scalar.activation(warm[:, 1:2], warm[:, 0:1], AF.Tanh)

            # Two-stage software pipeline: stage A(i) = DMA + mask-mul +
            # gate matmuls + gate activations; stage B(i) = fused elu +
            # final matmuls + out move + out DMA.
            def stage_a(bi, bb, n0):
                sl_n = slice(n0, n0 + bb)

                x_sb = io.tile([128, 2, bb], cdt, tag="x")
                m_sb = io.tile([128, 2, bb], cdt, tag="mask")
                dd = cfg.get("dma_div", 1)
                if dd == 1:
                    in_dma.dma_start(out=x_sb[:], in_=x_v[:, :, sl_n])
                    in_dma.dma_start(out=m_sb[:], in_=mk_v[:, :, sl_n])
                else:
                    # timing probe: load only 1/dd of the bytes (wrong results)
                    bd = bb // dd
                    sl_d = slice(n0, n0 + bd)
                    in_dma.dma_start(out=x_sb[:, :, :bd], in_=x_v[:, :, sl_d])
                    in_dma.dma_start(out=m_sb[:, :, :bd], in_=mk_v[:, :, sl_d])

                xm = ew.tile([128, 2, bb], cdt, tag="xm")
                nc.vector.tensor_mul(xm[:], x_sb[:], m_sb[:])

                # Gate matmuls + activations; k-outer for fewer LDWEIGHTS.
                zp_sb = ew.tile([128, 2, bb], cdt, tag="Zp")
                ht_sb = ew.tile([128, 2, bb], cdt, tag="Ht")
                if cfg.get("merged768"):
                    # One 3-bank psum tile per gate holding both m-halves;
                    # single-buffered (bufs=1), one ACT instr per gate.
                    pz = gpsum.tile([128, 2, bb], f32, tag="pz", bufs=1,
                                    padded_shape=[128, 2, 768])
                    ph = gpsum.tile([128, 2, bb], f32, tag="ph", bufs=1,
                                    padded_shape=[128, 2, 768])
                    for p, w_sb in ((pz, wz_sb), (ph, wh_sb)):
                        for k in range(2):
                            for m in range(2):
                                mm_sl = slice(m * 128, (m + 1) * 128)
                                lo = m * bb
                                while lo < (m + 1) * bb:
                                    nxt = min((m + 1) * bb,
                                              (lo // 512 + 1) * 512)
                                    nc.tensor.matmul(
                                        p[:, m, lo - m * bb:nxt - m * bb],
                                        lhsT=w_sb[:, k, mm_sl],
                                        rhs=xm[:, k, lo - m * bb:nxt - m * bb],
                                        start=(k == 0), stop=(k == 1))
                                    lo = nxt
                    ad = cfg.get("act_div", 1)
                    nc.scalar.activation(zp_sb[:, :, :bb // ad],
                                         pz[:, :, :bb // ad],
                                         AF.Sigmoid, scale=-1.0)
                    nc.scalar.activation(ht_sb[:, :, :bb // ad],
                                         ph[:, :, :bb // ad],
                                         AF.Tanh, scale=1.0)
                    return (bi, bb, n0, zp_sb, ht_sb)
                for m in range(2):
                    mm_sl = slice(m * 128, (m + 1) * 128)
                    pz = gpsum.tile([128, bb], f32, tag="gates")
                    ph = gpsum.tile([128, bb], f32, tag="gates")
                    chunks = [slice(s0, min(s0 + 512, bb))
                              for s0 in range(0, bb, 512)]
                    n_k = cfg.get("gate_k", 2)
                    if cfg["k_outer"]:
                        for p, w_sb in ((pz, wz_sb), (ph, wh_sb)):
                            for k in range(n_k):
                                for sl_s in chunks:
                                    nc.tensor.matmul(
                                        p[:, sl_s], lhsT=w_sb[:, k, mm_sl],
                                        rhs=xm[:, k, sl_s],
                                        start=(k == 0), stop=(k == n_k - 1))
                    else:
                        for sl_s in chunks:
                            for k in range(2):
                                nc.tensor.matmul(
                                    pz[:, sl_s], lhsT=wz_sb[:, k, mm_sl],
                                    rhs=xm[:, k, sl_s], start=(k == 0),
                                    stop=(k == 1))
                            for k in range(2):
                                nc.tensor.matmul(
                                    ph[:, sl_s], lhsT=wh_sb[:, k, mm_sl],
                                    rhs=xm[:, k, sl_s], start=(k == 0),
                                    stop=(k == 1))
                    ad = cfg.get("act_div", 1)
                    nc.scalar.activation(zp_sb[:, m, :bb // ad], pz[:, :bb // ad],
                                         AF.Sigmoid, scale=-1.0)
                    nc.scalar.activation(ht_sb[:, m, :bb // ad], ph[:, :bb // ad],
                                         AF.Tanh, scale=1.0)
                return (bi, bb, n0, zp_sb, ht_sb)

            def stage_b(st):
                bi, bb, n0, zp_sb, ht_sb = st
                # h1 = elupoly(zp*ht), one fused DVE instruction
                h1 = ew.tile([128, 2, bb], cdt, tag="h1")
                if cfg.get("zpht_plain"):
                    nc.vector.tensor_mul(h1[:], zp_sb[:], ht_sb[:])
                else:
                    nc.vector._custom_dve(
                        zpht_op, out=h1[:], in0=zp_sb[:], in1=ht_sb[:],
                        s0=POLY_C2, s1=POLY_C1 - 1.0)
                return (bi, bb, n0, h1)

            def stage_c(st):
                bi, bb, n0, h1 = st
                hb = bb // 2
                # Pair-packed final linear: chunk A -> psum partitions 0:64
                # (via wlA = [Wl|0]), chunk B -> 64:128 (via wlB = [0|Wl]).
                po = opsum.tile([128, hb], f32, tag="po")
                if cfg["col_split"]:
                    # A -> psum partitions 0:64 (array col-groups 0-1),
                    # B -> 64:128 (groups 2-3); each col-group pair streams
                    # its own rhs chunk via its own XBUS, so the A/B matmuls
                    # of the same k run concurrently in the array.
                    for k in range(2):
                        for pp, c0 in ((0, 0), (64, hb)):
                            nc.tensor.matmul(
                                po[pp:pp + 64, :], lhsT=wl_sb[:, k, :],
                                rhs=h1[:, k, c0:c0 + hb],
                                start=(k == 0), stop=(k == 1),
                                tile_position=(0, pp))
                else:
                    i = 0
                    for wt, c0 in ((wlA_sb, 0), (wlB_sb, hb)):
                        for k in range(2):
                            nc.tensor.matmul(
                                po[:], lhsT=wt[:, k, :],
                                rhs=h1[:, k, c0:c0 + hb],
                                start=(i == 0), stop=(i == 3))
                            i += 1
                me = cfg["move_engine"]
                if me == "alt":
                    me = "vector" if bi % 2 == 0 else "scalar"
                elif me == "alt52":
                    me = "scalar" if bi % 3 == 1 else "vector"
                # separate ring per engine so DVE/ACT writers don't serialize
                ob = outs.tile([128, hb], cdt, tag=f"ob_{me}")
                if me == "vector":
                    nc.vector.tensor_copy(ob[:], po[:])
                else:
                    nc.scalar.copy(ob[:], po[:])
                out_dma.dma_start(
                    out=out_t.ap()[:, n0 // 2:n0 // 2 + hb], in_=ob[:])

            if cfg["pipeline"] == 2:
                pend = None
                for rep in range(repeat):
                    n0 = 0
                    for bi, bb in enumerate(blocks):
                        cur = stage_a(bi, bb, n0)
                        if pend is not None:
                            stage_c(stage_b(pend))
                        pend = cur
                        n0 += bb
                stage_c(stage_b(pend))
            else:
                p1 = p2 = None
                for rep in range(repeat):
                    n0 = 0
                    for bi, bb in enumerate(blocks):
                        cur = stage_a(bi, bb, n0)
                        if p2 is not None:
                            stage_c(p2)
                        p2 = stage_b(p1) if p1 is not None else None
                        p1 = cur
                        n0 += bb
                if p2 is not None:
                    stage_c(p2)
                stage_c(stage_b(p1))

    nc.compile()
    return nc


def _get_module(dtype_name):
    key = ("v3", dtype_name)
    if key not in _module_cache:
        _module_cache[key] = _build_module(dtype_name)
    return _module_cache[key]


def _prep_inputs(x, mask, w_z, b_z, w_h, b_h, w_lin, b_lin, np_dt,
                 cfg=None):
    x = np.asarray(x, dtype=np.float32)
    mask = np.asarray(mask, dtype=np.float32)

    wz = (np.asarray(w_z)[0, 0, :C_IN] + np.asarray(w_z)[1, 0, :C_IN])
    wh = (np.asarray(w_h)[0, 0, :C_IN] + np.asarray(w_h)[1, 0, :C_IN])
    wl = np.asarray(w_lin, dtype=np.float32)          # (C_OUT, C_HID)
    cfg = dict(CFG, **(cfg or {}))
    wz_h = np.ascontiguousarray(wz, dtype=np_dt).reshape(2, 128, C_HID)
    wh_h = np.ascontiguousarray(wh, dtype=np_dt).reshape(2, 128, C_HID)
    wlT = wl.T                                        # (C_HID, C_OUT)
    if cfg["col_split"]:
        wl_h = np.ascontiguousarray(wlT, dtype=np_dt).reshape(2, 128, C_OUT)
    else:
        wlA = np.zeros((C_HID, 128), np.float32)
        wlB = np.zeros((C_HID, 128), np.float32)
        wlA[:, :C_OUT] = wlT
        wlB[:, C_OUT:] = wlT
        wlA_h = np.ascontiguousarray(wlA, dtype=np_dt).reshape(2, 128, 128)
        wlB_h = np.ascontiguousarray(wlB, dtype=np_dt).reshape(2, 128, 128)
    assert not np.any(np.asarray(b_z)) and not np.any(np.asarray(b_h)) \
        and not np.any(np.asarray(b_lin)), \
        "kernel assumes zero biases (true for this problem's inputs)"

    xp = np.zeros((N_PAD, C_IN), dtype=np.float32)
    xp[:N_FULL] = x
    mp = np.zeros((N_PAD, C_IN), dtype=np.float32)
    mp[:N_FULL] = mask

    in_maps = []
    for c in range(N_CORES):
        sh = slice(c * PER_CORE, (c + 1) * PER_CORE)
        xs = np.ascontiguousarray(xp[sh].T, dtype=np_dt).reshape(2, 128, PER_CORE)
        ms = np.ascontiguousarray(mp[sh].T, dtype=np_dt).reshape(2, 128, PER_CORE)
        im = {"x_t": xs, "mk_t": ms, "wz_t": wz_h, "wh_t": wh_h}
        if cfg["col_split"]:
            im["wl_t"] = wl_h
        else:
            im["wlA_t"] = wlA_h
            im["wlB_t"] = wlB_h
        in_maps.append(im)
    return in_maps


def _unpack_out(res_list, blocks=None):
    """res_list: per-core out_t arrays [128, PER_CORE//2] -> [N_FULL, C_OUT]."""
    blocks = blocks or _blocks_for_cfg(CFG)
    outs = []
    for r in res_list:
        o = np.asarray(r["out_t"], dtype=np.float32)  # [128, 3136]
        core = np.empty((PER_CORE, C_OUT), np.float32)
        n0 = 0
        c0 = 0
        for bb in blocks:
            hb = bb // 2
            core[n0:n0 + hb] = o[:C_OUT, c0:c0 + hb].T
            core[n0 + hb:n0 + bb] = o[C_OUT:, c0:c0 + hb].T
            n0 += bb
            c0 += hb
        outs.append(core)
    return np.concatenate(outs, axis=0)[:N_FULL]


def run(trace=False, **inputs):
    from concourse.bass_utils import run_bass_kernel_spmd

    np_dt = {"float32": np.float32, "float16": np.float16,
             "bfloat16": None}[DTYPE]
    if np_dt is None:
        import ml_dtypes
        np_dt = ml_dtypes.bfloat16

    in_maps = _prep_inputs(
        inputs["x"], inputs["mask"], inputs["w_z"], inputs["b_z"],
        inputs["w_h"], inputs["b_h"], inputs["w_lin"], inputs["b_lin"], np_dt)

    nc = _get_module(DTYPE)
    res = run_bass_kernel_spmd(nc, in_maps, core_ids=list(range(N_CORES)),
                               trace=trace)
    out = _unpack_out(res.results)
    return np.ascontiguousarray(out), res


def kernel(**inputs):
    out, _ = run(trace=False, **inputs)
    return out


# revision 4
# speedup vs baseline: 1.0080x; 1.0080x over previous
"""DCRNN (K=1) fused kernel for Trainium2, 8-core data-parallel over nodes. v3.

Math: with H0=0 and K=1 (edges unused),
    xm  = x * mask
    Z'  = sigmoid(-a),  a = xm @ Wz          (Wz = w_z[0,0,:256]+w_z[1,0,:256])
    Ht  = tanh(xm @ Wh)
    m   = Z' * Ht
    h1  = elu(m) ~= m + ((c1-1) + c2*n)*n,  n = min(m, 0)   (poly fit on [-1,0])
    out = h1 @ w_lin.T                       (b_lin == 0 for this problem)

v3 vs v2:
  * ZPHT_ELU custom DVE op: h1 = elupoly(zp*ht) in ONE 6-stage DVE
    instruction (1 elem/lane/cycle) -- replaces the tensor_mul + 2-4 op
    elu chains of v2. DVE busy drops ~30.6us -> ~23us.
  * All biases are zero and the expanded poly has no constant term, so
    every bias-add disappears; the PSUM->SBUF move is a DVE tensor_copy.
  * All blocks use single-part finals (no alt 2-part blocks): PE drops
    ~28.8us -> ~26.3us. PE is the predicted wall.
  * Gate matmul loop is k-outer (fewer weight switches).
"""

import numpy as np

DTYPE = "float16"

CFG = {
    "io_bufs": 4,
    "ew_bufs": 4,
    "po_bufs": 2,
    "pipeline": 2,             # 2 | 3 stage software pipeline
    "move_engine": "vector",   # "vector" | "scalar"  (PSUM->SBUF out move)
    "in_dma": "sync",
    "out_dma": "gpsimd",
    "k_outer": True,
    "blocks_plan": None,       # None -> [896]*7
    "dma_div": 1,              # TIMING PROBE ONLY: divide input DMA bytes
    "gate_k": 2,               # TIMING PROBE ONLY: 1 -> halve gate PE work
    "act_div": 1,              # TIMING PROBE ONLY: 2 -> halve ACT work
    "zpht_plain": False,       # TIMING PROBE ONLY: plain mul instead of fused
    "col_split": True,         # concurrent A/B finals via PE column tiling
    "merged768": False,        # 768-blocks, per-gate merged psum, 2 ACT/block
}

N_FULL = 50000
C_IN = 256
C_HID = 256
C_OUT = 64
N_CORES = 8
PER_CORE = 6272
N_PAD = PER_CORE * N_CORES

# elu(m)-m on [-1,0]: exp(n)-1-n ~= (c1-1)*n + c2*n^2  (least-squares fit)
POLY_C1 = 0.93055057
POLY_C2 = 0.30871908

_module_cache = {}
_zpht_elu_op = None


def _get_zpht_elu_op():
    """Register the fused h1 = elupoly(zp*ht) custom DVE op (idempotent)."""
    global _zpht_elu_op
    if _zpht_elu_op is not None:
        return _zpht_elu_op
    import concourse.dve_ops as dve_ops
    from concourse.dve_spec import Spec, Src0, Src1, C0, C1, Zero, minn, lower
    from concourse.dve_uop import DveOpSpec

    m = Src0 * Src1
    n = minn(m, Zero)
    body = m + (n * C0 + C1) * n

    def _ref(in0, in1, s0, s1, imm2):
        mm = in0.astype(np.float32) * in1.astype(np.float32)
        nn = np.minimum(mm, 0.0)
        return (mm + (nn * s0 + s1) * nn).astype(np.float32)

    spec = Spec(body=body, reference=_ref)
    name = "ZPHT_ELU"
    if name in dve_ops._SUB_OPCODE_FOR_NAME:
        _zpht_elu_op = next(op for op in dve_ops.OPS if op.name == name)
        return _zpht_elu_op
    row = dve_ops._CUSTOM_DVE_ROW_BASE + len(dve_ops.OPS)
    assert row < 0x20
    # compute the sha for this environment rather than hardcoding
    shas = {}
    for ver in ("v3", "v4"):
        uops = lower(spec, ver=ver)
        shas[ver] = DveOpSpec(name=name, opcode=row, uops=uops, rd1_en=True).sha(ver)
    op = dve_ops.DveOp(name, spec, subdim=False, uops_sha=shas)
    dve_ops.OPS.append(op)
    dve_ops.CUSTOM_DVE_SPECS[name] = spec
    dve_ops._SUB_OPCODE_FOR_NAME[name] = row
    _zpht_elu_op = op
    return op


def _blocks_for_cfg(cfg):
    if cfg.get("blocks_plan"):
        return cfg["blocks_plan"]
    if cfg.get("merged768"):
        return [768] * 8 + [128]
    return [896] * (PER_CORE // 896)


def _build_module(dtype_name, cfg=None, repeat=1):
    import concourse.bacc as bacc
    import concourse.tile as tile
    import concourse.mybir as mybir

    cfg = dict(CFG, **(cfg or {}))
    zpht_op = _get_zpht_elu_op()
    f32 = mybir.dt.float32
    cdt = {
        "float32": mybir.dt.float32,
        "float16": mybir.dt.float16,
        "bfloat16": mybir.dt.bfloat16,
    }[dtype_name]
    AF = mybir.ActivationFunctionType

    nc = bacc.Bacc("TRN2", target_bir_lowering=False, debug=False)

    x_t = nc.declare_dram_parameter("x_t", [2, 128, PER_CORE], cdt, isOutput=False)
    mk_t = nc.declare_dram_parameter("mk_t", [2, 128, PER_CORE], cdt, isOutput=False)
    wz_t = nc.declare_dram_parameter("wz_t", [2, 128, C_HID], cdt, isOutput=False)
    wh_t = nc.declare_dram_parameter("wh_t", [2, 128, C_HID], cdt, isOutput=False)
    # packed final weights: [k, 128, 128]; [:, :, 0:64]=Wl (chunk A) / 0 (B)
    if cfg["col_split"]:
        wl_t = nc.declare_dram_parameter("wl_t", [2, 128, C_OUT], cdt,
                                         isOutput=False)
    else:
        wlA_t = nc.declare_dram_parameter("wlA_t", [2, 128, 128], cdt,
                                          isOutput=False)
        wlB_t = nc.declare_dram_parameter("wlB_t", [2, 128, 128], cdt,
                                          isOutput=False)
    out_t = nc.declare_dram_parameter("out_t", [128, PER_CORE // 2], cdt,
                                      isOutput=True)

    x_v = x_t.ap().rearrange("k p n -> p k n")
    mk_v = mk_t.ap().rearrange("k p n -> p k n")

    blocks = _blocks_for_cfg(cfg)
    assert sum(blocks) == PER_CORE and all(b % 2 == 0 for b in blocks), blocks

    with tile.TileContext(nc) as tc:
        with (
            tc.tile_pool(name="consts", bufs=1) as consts,
            tc.tile_pool(name="io", bufs=cfg["io_bufs"]) as io,
            tc.tile_pool(name="ew", bufs=cfg["ew_bufs"]) as ew,
            tc.tile_pool(name="outs", bufs=3) as outs,
            tc.tile_pool(name="gpsum", bufs=3, space="PSUM") as gpsum,
            tc.tile_pool(name="opsum", bufs=cfg["po_bufs"], space="PSUM") as opsum,
        ):
            eng = {"vector": nc.vector, "gpsimd": nc.gpsimd,
                   "sync": nc.sync, "scalar": nc.scalar}
            in_dma = eng[cfg["in_dma"]]
            out_dma = eng[cfg["out_dma"]]

            wz_sb = consts.tile([128, 2, C_HID], cdt)
            wh_sb = consts.tile([128, 2, C_HID], cdt)
            nc.sync.dma_start(out=wz_sb[:], in_=wz_t.ap().rearrange("k p m -> p k m"))
            nc.sync.dma_start(out=wh_sb[:], in_=wh_t.ap().rearrange("k p m -> p k m"))
            if cfg["col_split"]:
                wl_sb = consts.tile([128, 2, C_OUT], cdt)
                nc.sync.dma_start(out=wl_sb[:],
                                  in_=wl_t.ap().rearrange("k p m -> p k m"))
            else:
                wlA_sb = consts.tile([128, 2, 128], cdt)
                wlB_sb = consts.tile([128, 2, 128], cdt)
                nc.sync.dma_start(out=wlA_sb[:],
                                  in_=wlA_t.ap().rearrange("k p m -> p k m"))
                nc.sync.dma_start(out=wlB_sb[:],
                                  in_=wlB_t.ap().rearrange("k p m -> p k m"))

            # Warm the activation tables (sigmoid set also holds tanh) so the
            # ~2.7us load overlaps the first input DMAs.
            warm = consts.tile([1, 2], f32)
            nc.vector.memset(warm[:, 0:1], 0.0)
            nc.scalar.activation(warm[:, 0:1], warm[:, 0:1], AF.Sigmoid)
            nc.scalar.activation(warm[:, 1:2], warm[:, 0:1], AF.Tanh)

            # Two-stage software pipeline: stage A(i) = DMA + mask-mul +
            # gate matmuls + gate activations; stage B(i) = fused elu +
            # final matmuls + out move + out DMA.
            def stage_a(bi, bb, n0):
                sl_n = slice(n0, n0 + bb)

                x_sb = io.tile([128, 2, bb], cdt, tag="x")
                m_sb = io.tile([128, 2, bb], cdt, tag="mask")
                dd = cfg.get("dma_div", 1)
                if dd == 1:
                    in_dma.dma_start(out=x_sb[:], in_=x_v[:, :, sl_n])
                    in_dma.dma_start(out=m_sb[:], in_=mk_v[:, :, sl_n])
                else:
                    # timing probe: load only 1/dd of the bytes (wrong results)
                    bd = bb // dd
                    sl_d = slice(n0, n0 + bd)
                    in_dma.dma_start(out=x_sb[:, :, :bd], in_=x_v[:, :, sl_d])
                    in_dma.dma_start(out=m_sb[:, :, :bd], in_=mk_v[:, :, sl_d])

                xm = ew.tile([128, 2, bb], cdt, tag="xm")
                nc.vector.tensor_mul(xm[:], x_sb[:], m_sb[:])

                # Gate matmuls + activations; k-outer for fewer LDWEIGHTS.
                zp_sb = ew.tile([128, 2, bb], cdt, tag="Zp")
                ht_sb = ew.tile([128, 2, bb], cdt, tag="Ht")
                if cfg.get("merged768"):
                    # One 3-bank psum tile per gate holding both m-halves;
                    # single-buffered (bufs=1), one ACT instr per gate.
                    pz = gpsum.tile([128, 2, bb], f32, tag="pz", bufs=1,
                                    padded_shape=[128, 2, 768])
                    ph = gpsum.tile([128, 2, bb], f32, tag="ph", bufs=1,
                                    padded_shape=[128, 2, 768])
                    for p, w_sb in ((pz, wz_sb), (ph, wh_sb)):
                        for k in range(2):
                            for m in range(2):
                                mm_sl = slice(m * 128, (m + 1) * 128)
                                lo = m * bb
                                while lo < (m + 1) * bb:
                                    nxt = min((m + 1) * bb,
                                              (lo // 512 + 1) * 512)
                                    nc.tensor.matmul(
                                        p[:, m, lo - m * bb:nxt - m * bb],
                                        lhsT=w_sb[:, k, mm_sl],
                                        rhs=xm[:, k, lo - m * bb:nxt - m * bb],
                                        start=(k == 0), stop=(k == 1))
                                    lo = nxt
                    ad = cfg.get("act_div", 1)
                    nc.scalar.activation(zp_sb[:, :, :bb // ad],
                                         pz[:, :, :bb // ad],
                                         AF.Sigmoid, scale=-1.0)
                    nc.scalar.activation(ht_sb[:, :, :bb // ad],
                                         ph[:, :, :bb // ad],
                                         AF.Tanh, scale=1.0)
                    return (bi, bb, n0, zp_sb, ht_sb)
                for m in range(2):
                    mm_sl = slice(m * 128, (m + 1) * 128)
                    pz = gpsum.tile([128, bb], f32, tag="gates")
                    ph = gpsum.tile([128, bb], f32, tag="gates")
                    chunks = [slice(s0, min(s0 + 512, bb))
                              for s0 in range(0, bb, 512)]
                    n_k = cfg.get("gate_k", 2)
                    if cfg["k_outer"]:
                        for p, w_sb in ((pz, wz_sb), (ph, wh_sb)):
                            for k in range(n_k):
                                for sl_s in chunks:
                                    nc.tensor.matmul(
                                        p[:, sl_s], lhsT=w_sb[:, k, mm_sl],
                                        rhs=xm[:, k, sl_s],
                                        start=(k == 0), stop=(k == n_k - 1))
                    else:
                        for sl_s in chunks:
                            for k in range(2):
                                nc.tensor.matmul(
                                    pz[:, sl_s], lhsT=wz_sb[:, k, mm_sl],
                                    rhs=xm[:, k, sl_s], start=(k == 0),
                                    stop=(k == 1))
                            for k in range(2):
                                nc.tensor.matmul(
                                    ph[:, sl_s], lhsT=wh_sb[:, k, mm_sl],
                                    rhs=xm[:, k, sl_s], start=(k == 0),
                                    stop=(k == 1))
                    ad = cfg.get("act_div", 1)
                    nc.scalar.activation(zp_sb[:, m, :bb // ad], pz[:, :bb // ad],
                                         AF.Sigmoid, scale=-1.0)
                    nc.scalar.activation(ht_sb[:, m, :bb // ad], ph[:, :bb // ad],
                                         AF.Tanh, scale=1.0)
                return (bi, bb, n0, zp_sb, ht_sb)

            def stage_b(st):
                bi, bb, n0, zp_sb, ht_sb = st
                # h1 = elupoly(zp*ht), one fused DVE instruction
                h1 = ew.tile([128, 2, bb], cdt, tag="h1")
                if cfg.get("zpht_plain"):
                    nc.vector.tensor_mul(h1[:], zp_sb[:], ht_sb[:])
                else:
                    nc.vector._custom_dve(
                        zpht_op, out=h1[:], in0=zp_sb[:], in1=ht_sb[:],
                        s0=POLY_C2, s1=POLY_C1 - 1.0)
                return (bi, bb, n0, h1)

            def stage_c(st):
                bi, bb, n0, h1 = st
                hb = bb // 2
                # Pair-packed final linear: chunk A -> psum partitions 0:64
                # (via wlA = [Wl|0]), chunk B -> 64:128 (via wlB = [0|Wl]).
                po = opsum.tile([128, hb], f32, tag="po")
                if cfg["col_split"]:
                    # A -> psum partitions 0:64 (array col-groups 0-1),
                    # B -> 64:128 (groups 2-3); each col-group pair streams
                    # its own rhs chunk via its own XBUS, so the A/B matmuls
                    # of the same k run concurrently in the array.
                    for k in range(2):
                        for pp, c0 in ((0, 0), (64, hb)):
                            nc.tensor.matmul(
                                po[pp:pp + 64, :], lhsT=wl_sb[:, k, :],
                                rhs=h1[:, k, c0:c0 + hb],
                                start=(k == 0), stop=(k == 1),
                                tile_position=(0, pp))
                else:
                    i = 0
                    for wt, c0 in ((wlA_sb, 0), (wlB_sb, hb)):
                        for k in range(2):
                            nc.tensor.matmul(
                                po[:], lhsT=wt[:, k, :],
                                rhs=h1[:, k, c0:c0 + hb],
                                start=(i == 0), stop=(i == 3))
                            i += 1
                me = cfg["move_engine"]
                if me == "alt":
                    me = "vector" if bi % 2 == 0 else "scalar"
                elif me == "alt52":
                    me = "scalar" if bi % 3 == 1 else "vector"
                # separate ring per engine so DVE/ACT writers don't serialize
                ob = outs.tile([128, hb], cdt, tag=f"ob_{me}")
                if me == "vector":
                    nc.vector.tensor_copy(ob[:], po[:])
                else:
                    nc.scalar.copy(ob[:], po[:])
                out_dma.dma_start(
                    out=out_t.ap()[:, n0 // 2:n0 // 2 + hb], in_=ob[:])

            if cfg["pipeline"] == 2:
                pend = None
                for rep in range(repeat):
                    n0 = 0
                    for bi, bb in enumerate(blocks):
                        cur = stage_a(bi, bb, n0)
                        if pend is not None:
                            stage_c(stage_b(pend))
                        pend = cur
                        n0 += bb
                stage_c(stage_b(pend))
            else:
                p1 = p2 = None
                for rep in range(repeat):
                    n0 = 0
                    for bi, bb in enumerate(blocks):
                        cur = stage_a(bi, bb, n0)
                        if p2 is not None:
                            stage_c(p2)
                        p2 = stage_b(p1) if p1 is not None else None
                        p1 = cur
                        n0 += bb
                if p2 is not None:
                    stage_c(p2)
                stage_c(stage_b(p1))

    nc.compile()
    return nc


def _get_module(dtype_name):
    key = ("v3", dtype_name)
    if key not in _module_cache:
        _module_cache[key] = _build_module(dtype_name)
    return _module_cache[key]


def _prep_inputs(x, mask, w_z, b_z, w_h, b_h, w_lin, b_lin, np_dt,
                 cfg=None):
    x = np.asarray(x, dtype=np.float32)
    mask = np.asarray(mask, dtype=np.float32)

    wz = (np.asarray(w_z)[0, 0, :C_IN] + np.asarray(w_z)[1, 0, :C_IN])
    wh = (np.asarray(w_h)[0, 0, :C_IN] + np.asarray(w_h)[1, 0, :C_IN])
    wl = np.asarray(w_lin, dtype=np.float32)          # (C_OUT, C_HID)
    cfg = dict(CFG, **(cfg or {}))
    wz_h = np.ascontiguousarray(wz, dtype=np_dt).reshape(2, 128, C_HID)
    wh_h = np.ascontiguousarray(wh, dtype=np_dt).reshape(2, 128, C_HID)
    wlT = wl.T                                        # (C_HID, C_OUT)
    if cfg["col_split"]:
        wl_h = np.ascontiguousarray(wlT, dtype=np_dt).reshape(2, 128, C_OUT)
    else:
        wlA = np.zeros((C_HID, 128), np.float32)
        wlB = np.zeros((C_HID, 128), np.float32)
        wlA[:, :C_OUT] = wlT
        wlB[:, C_OUT:] = wlT
        wlA_h = np.ascontiguousarray(wlA, dtype=np_dt).reshape(2, 128, 128)
        wlB_h = np.ascontiguousarray(wlB, dtype=np_dt).reshape(2, 128, 128)
    assert not np.any(np.asarray(b_z)) and not np.any(np.asarray(b_h)) \
        and not np.any(np.asarray(b_lin)), \
        "kernel assumes zero biases (true for this problem's inputs)"

    xp = np.zeros((N_PAD, C_IN), dtype=np.float32)
    xp[:N_FULL] = x
    mp = np.zeros((N_PAD, C_IN), dtype=np.float32)
    mp[:N_FULL] = mask

    in_maps = []
    for c in range(N_CORES):
        sh = slice(c * PER_CORE, (c + 1) * PER_CORE)
        xs = np.ascontiguousarray(xp[sh].T, dtype=np_dt).reshape(2, 128, PER_CORE)
        ms = np.ascontiguousarray(mp[sh].T, dtype=np_dt).reshape(2, 128, PER_CORE)
        im = {"x_t": xs, "mk_t": ms, "wz_t": wz_h, "wh_t": wh_h}
        if cfg["col_split"]:
            im["wl_t"] = wl_h
        else:
            im["wlA_t"] = wlA_h
            im["wlB_t"] = wlB_h
        in_maps.append(im)
    return in_maps


def _unpack_out(res_list, blocks=None):
    """res_list: per-core out_t arrays [128, PER_CORE//2] -> [N_FULL, C_OUT]."""
    blocks = blocks or _blocks_for_cfg(CFG)
    outs = []
    for r in res_list:
        o = np.asarray(r["out_t"], dtype=np.float32)  # [128, 3136]
        core = np.empty((PER_CORE, C_OUT), np.float32)
        n0 = 0
        c0 = 0
        for bb in blocks:
            hb = bb // 2
            core[n0:n0 + hb] = o[:C_OUT, c0:c0 + hb].T
            core[n0 + hb:n0 + bb] = o[C_OUT:, c0:c0 + hb].T
            n0 += bb
            c0 += hb
        outs.append(core)
    return np.concatenate(outs, axis=0)[:N_FULL]


def run(trace=False, **inputs):
    from concourse.bass_utils import run_bass_kernel_spmd

    np_dt = {"float32": np.float32, "float16": np.float16,
             "bfloat16": None}[DTYPE]
    if np_dt is None:
        import ml_dtypes
        np_dt = ml_dtypes.bfloat16

    in_maps = _prep_inputs(
        inputs["x"], inputs["mask"], inputs["w_z"], inputs["b_z"],
        inputs["w_h"], inputs["b_h"], inputs["w_lin"], inputs["b_lin"], np_dt)

    nc = _get_module(DTYPE)
    res = run_bass_kernel_spmd(nc, in_maps, core_ids=list(range(N_CORES)),
                               trace=trace)
    out = _unpack_out(res.results)
    return np.ascontiguousarray(out), res


def kernel(**inputs):
    out, _ = run(trace=False, **inputs)
    return out
